# revision 31
# baseline (speedup 1.0000x reference)
"""HRA (Householder Reflection Adaptation) forward kernel for Trainium2.

Math: out = x @ Q with Q = prod_i (I - 2 u_i u_i^T), u_i = normalized columns
of hra_u [4096, 8].  Using the compact WY representation:
    Q = I - U T U^T      (T upper-triangular 8x8, diag=2)
    out = x - (x @ A) @ U^T,   A = U @ T

Sharding: data-parallel over rows. x [4,2048,4096] -> [8192, 4096]; each of
8 cores gets 1024 contiguous rows.

Layout/precision strategy (HBM-bandwidth / PE-instruction bound):
  * device I/O is fp16 (tolerance is 2e-2; fp16 keeps rel err ~1e-4),
    halving HBM traffic vs f32: 16.8 MB/core round trip.
  * the host uploads x TRANSPOSED (x^T [4096, 1024] per core), so the
    projection matmuls P^T[8,r] += A_c^T @ xT_c run directly on DMA'd
    tiles -- no PE transposes and no PSUM->SBUF copy pass at all.
  * updates stay transposed: outT_c = xT_c - U_c @ P^T, subtracted in
    place in SBUF; the host transposes the fp16 result back.

Schedule (monolithic; input phase = DMA+PE only, then the tail):
  32 chunk DMAs stream in while the PE accumulates P^T into two
  alternating PSUM regions (folded per row-half at the phase boundary
  so the first update matmul waits only on the half-0 fold); the tail
  then runs per chunk: 2 update matmuls -> PSUM, drain to fp16 SBUF
  rotated across DVE (direct psum-subtract), ACT convert + DVE
  fp16-subtract, and ACT convert + Pool fp16-subtract, then DMA-out.
  Out-triggers are emitted per 4-chunk group ordered fast-drains-first
  so a slow Pool drain never head-of-line blocks the SP trigger queue,
  and each out-DMA writes a 2-chunk superline (4 KB/partition; 2 KB
  lines cap ~250 GB/s, 4 KB+ sustain ~400).  All PSUM tiles share one
  4-slot ring: the projection partials' slots recycle into the tail
  rotation after the fold, giving the PE 4-deep runahead over the
  drain engines.

Measured constraints this design works around (from neuron-profile):
  * a power governor clamps the clock (PE 376 -> 625 ns per 512-col
    matmul) once the drain engines light up alongside PE+DMA; every
    overlapped schedule tried trips it identically, so the monolithic
    shape with the fewest total PE instructions wins.
  * matmul output is hard-capped at one PSUM bank (512 f32) by the
    ISA, and the toolchain emits LDWEIGHTS per matmul, so the 128
    data matmuls (~473-813 ns each) are the structural floor.
"""

import os
import sys

for _p in ("/opt/trn_rl_repo", "/root/.axon_site", "/root/.axon_site/_ro/trn_rl_repo",
           "/root/.axon_site/_ro/pypackages"):
    if os.path.isdir(_p) and _p not in sys.path:
        sys.path.append(_p)

import numpy as np

import concourse.bass as bass
import concourse.mybir as mybir
import concourse.tile as tile
from concourse import bacc
from concourse.bass_utils import run_bass_kernel_spmd

B, S, D, R = 4, 2048, 4096, 8
N_CORES = 8
ROWS = B * S                      # 8192
RPC = ROWS // N_CORES             # 1024 rows per core
P = 128
DC = D // P                       # 32 d-chunks
H = RPC // 2                      # 512 rows per half (PSUM bank f32 size)

F32 = mybir.dt.float32
F16 = mybir.dt.float16

_CACHE = {}


def _householder_wy(hra_u: np.ndarray):
    """Return (A, UT) with out = x - (x @ A) @ UT."""
    u = hra_u.astype(np.float32)
    u = u / np.linalg.norm(u, axis=0, keepdims=True)
    U = u.astype(np.float64)
    T = np.zeros((R, R), np.float64)
    for k in range(R):
        T[k, k] = 2.0
        if k:
            T[:k, k] = -2.0 * (T[:k, :k] @ (U[:, :k].T @ U[:, k]))
    A = (U @ T).astype(np.float32)          # [D, R]
    return A, np.ascontiguousarray(u.T)     # [R, D]


# tail drain rotation per 8 chunks: 0=DVE direct psum-sub,
# 1=ACT convert + DVE fp16 sub, 2=ACT convert + Pool fp16 sub
_TAIL = [0, 1, 2, 0, 1, 0, 1, 2]


def _build_program():
    nc = bacc.Bacc(trn_type="TRN2")
    xt = nc.dram_tensor("xt", (D, RPC), F16, kind="ExternalInput")
    a = nc.dram_tensor("a", (P, DC * R), F16, kind="ExternalInput")
    ut = nc.dram_tensor("ut", (R, D), F16, kind="ExternalInput")
    # out is laid out as 2-chunk superlines [(g p), (j r)] so each DMA-out
    # writes 4 KB contiguous per partition (2 KB lines cap ~250 GB/s, 4 KB+
    # sustain ~370-430); the host unpacks.
    out = nc.dram_tensor("out", (D // 2, 2 * RPC), F16, kind="ExternalOutput")

    xtd = xt.rearrange("(c p) r -> p c r", p=P)   # [128, DC, RPC]
    otd = out.rearrange("(g p) (j r) -> p g j r", p=P, j=2)

    with tile.TileContext(nc) as tc:
        with (
            tc.tile_pool(name="const", bufs=1) as const,
            tc.tile_pool(name="upd", bufs=4) as upd_pool,
            tc.tile_pool(name="pso", bufs=4, space="PSUM") as pso_pool,
        ):
            a_sb = const.tile([P, DC * R], F16)
            nc.sync.dma_start(a_sb, a[:, :])
            ut_sb = const.tile([R, D], F16)
            nc.sync.dma_start(ut_sb, ut[:, :])

            xall = const.tile([P, DC, RPC], F16)
            nc.sync.dma_start(xall[:, 0, :], xtd[:, 0, :])

            # tiny PE warm-up: observe each const DMA once (one sync-wait
            # per LDWEIGHTS) and keep the PE awake during the DMA fill
            # without delaying the first projection matmuls.
            warm = pso_pool.tile([P, 2, H], F32, tag="ps_o")
            nc.tensor.matmul(warm[:R, 0, :256], a_sb[:, :R], a_sb[:, :256],
                             start=True, stop=True)
            for _ in range(4):
                nc.tensor.matmul(warm[:, 0, :P], ut_sb[:, :P], ut_sb[:, :P],
                                 start=True, stop=True)

            for c in range(1, DC):
                nc.sync.dma_start(xall[:, c, :], xtd[:, c, :])

            # both partials live in the pso ring; their slots recycle into
            # the tail rotation after the fold, ramping runahead 2 -> 4
            ps_p1 = pso_pool.tile([R, 2, H], F32, tag="ps_o")
            ps_p2 = pso_pool.tile([R, 2, H], F32, tag="ps_o")
            pt = const.tile([R, 2, H], F16)

            # projection: P^T[8, RPC] += A_c^T @ xT_c, accumulated into two
            # alternating PSUM regions (even/odd chunks) so consecutive
            # matmuls never hit the same bank's accumulate turnaround; the
            # two partials are folded by one DVE add at pt time.
            for c in range(DC):
                psp = ps_p1 if c % 2 == 0 else ps_p2
                for h in range(2):
                    nc.tensor.matmul(
                        psp[:, h, :],
                        a_sb[:, c * R:(c + 1) * R],
                        xall[:, c, h * H:(h + 1) * H],
                        start=(c < 2),
                        stop=(c >= DC - 2),
                    )
            # fold per half: the first update matmul (h0) only waits on
            # the h0 fold, shaving ~1us off the phase-boundary bubble
            p2s = const.tile([R, 2, H], F16)
            for h in range(2):
                nc.scalar.copy(p2s[:, h, :], ps_p2[:, h, :])
                nc.vector.tensor_add(pt[:, h, :], ps_p1[:, h, :],
                                     p2s[:, h, :])

            # tail: outT_c = xT_c - U_c @ P^T in place, then DMA out
            for c in range(DC):
                ps_o = pso_pool.tile([P, 2, H], F32, tag="ps_o")
                for h in range(2):
                    nc.tensor.matmul(
                        ps_o[:, h, :],
                        ut_sb[:, c * P:(c + 1) * P],
                        pt[:, h, :],
                        start=True,
                        stop=True,
                    )
                xc = xall[:, c, :]
                kind = _TAIL[c % 8]
                if kind == 0:
                    nc.vector.tensor_sub(xc, xc, ps_o)
                else:
                    u_sb = upd_pool.tile([P, 2, H], F16, tag="upd")
                    nc.scalar.copy(u_sb, ps_o)
                    if kind == 1:
                        nc.vector.tensor_sub(xc, xc, u_sb)
                    else:
                        nc.gpsimd.tensor_sub(xc, xc, u_sb)
                # flush 2-chunk superline triggers per 4-block, fastest
                # drain-pairs first, so a slow Pool drain never head-of-line
                # blocks the in-order SP trigger queue
                if c % 4 == 3:
                    pairs = [(c - 3) // 2, (c - 1) // 2]
                    pairs.sort(key=lambda g: max(_TAIL[(2 * g) % 8],
                                                 _TAIL[(2 * g + 1) % 8]))
                    for g in pairs:
                        nc.sync.dma_start(otd[:, g, :, :],
                                          xall[:, 2 * g:2 * g + 2, :])

    nc.compile()
    return nc


def _get_program():
    if "nc" not in _CACHE:
        _CACHE["nc"] = _build_program()
    return _CACHE["nc"]


def kernel(input, hra_u, **run_kwargs):
    input = np.asarray(input, dtype=np.float32)
    hra_u = np.asarray(hra_u, dtype=np.float32)

    A, UT = _householder_wy(hra_u)
    # pack A [D, R] so partition p holds A[c*128+p, :] at free offset c*R
    a_packed = np.ascontiguousarray(
        A.reshape(DC, P, R).transpose(1, 0, 2).reshape(P, DC * R)
    ).astype(np.float16)
    ut_f16 = UT.astype(np.float16)

    x_flat = input.reshape(ROWS, D)
    in_maps = [
        {
            "xt": x_flat[c * RPC:(c + 1) * RPC].T.astype(np.float16),
            "a": a_packed,
            "ut": ut_f16,
        }
        for c in range(N_CORES)
    ]

    nc = _get_program()
    res = run_bass_kernel_spmd(nc, in_maps, core_ids=list(range(N_CORES)),
                               **run_kwargs)
    out = np.empty((ROWS, D), dtype=np.float32)
    for c in range(N_CORES):
        o = res.results[c]["out"].reshape(DC // 2, P, 2, RPC)
        out[c * RPC:(c + 1) * RPC] = (
            o.transpose(0, 2, 1, 3).reshape(D, RPC).astype(np.float32).T
        )
    if run_kwargs:
        kernel.last_results = res
    return out.reshape(B, S, D)


# revision 32
# speedup vs baseline: 1.0763x; 1.0763x over previous
"""HRA (Householder Reflection Adaptation) forward kernel for Trainium2.

Math: out = x @ Q with Q = prod_i (I - 2 u_i u_i^T), u_i = normalized columns
of hra_u [4096, 8].  Using the compact WY representation:
    Q = I - U T U^T      (T upper-triangular 8x8, diag=2)
    out = x - (x @ A) @ U^T,   A = U @ T

Sharding: data-parallel over rows. x [4,2048,4096] -> [8192, 4096]; each of
8 cores gets 1024 contiguous rows.

Layout/precision strategy (HBM-bandwidth / PE-instruction bound):
  * device I/O is fp16 (tolerance is 2e-2; fp16 keeps rel err ~1e-4),
    halving HBM traffic vs f32: 16.8 MB/core round trip.
  * the host uploads x TRANSPOSED (x^T [4096, 1024] per core), so the
    projection matmuls P^T[8,r] += A_c^T @ xT_c run directly on DMA'd
    tiles -- no PE transposes and no PSUM->SBUF copy pass at all.
  * updates stay transposed: outT_c = xT_c - U_c @ P^T, subtracted in
    place in SBUF; the host transposes the fp16 result back.

Schedule (monolithic; input phase = DMA+PE only, then the tail):
  32 chunk DMAs stream in while the PE accumulates P^T into two
  alternating PSUM regions (folded per row-half at the phase boundary
  so the first update matmul waits only on the half-0 fold); the tail
  then runs per chunk: 2 update matmuls -> PSUM, drain to fp16 SBUF
  rotated across DVE (direct psum-subtract), ACT convert + DVE
  fp16-subtract, and ACT convert + Pool fp16-subtract, then DMA-out.
  Out-triggers are emitted per 4-chunk group ordered fast-drains-first
  so a slow Pool drain never head-of-line blocks the SP trigger queue,
  and each out-DMA writes a 2-chunk superline (4 KB/partition; 2 KB
  lines cap ~250 GB/s, 4 KB+ sustain ~400).  All PSUM tiles share one
  4-slot ring: the projection partials' slots recycle into the tail
  rotation after the fold, giving the PE 4-deep runahead over the
  drain engines.

Measured constraints this design works around (from neuron-profile):
  * a power governor clamps the clock (PE 376 -> 625 ns per 512-col
    matmul) once the drain engines light up alongside PE+DMA; every
    overlapped schedule tried trips it identically, so the monolithic
    shape with the fewest total PE instructions wins.
  * matmul output is hard-capped at one PSUM bank (512 f32) by the
    ISA, and the toolchain emits LDWEIGHTS per matmul, so the 128
    data matmuls (~473-813 ns each) are the structural floor.
"""

import os
import sys

for _p in ("/opt/trn_rl_repo", "/root/.axon_site", "/root/.axon_site/_ro/trn_rl_repo",
           "/root/.axon_site/_ro/pypackages"):
    if os.path.isdir(_p) and _p not in sys.path:
        sys.path.append(_p)

import numpy as np

import concourse.bass as bass
import concourse.mybir as mybir
import concourse.tile as tile
from concourse import bacc
from concourse.bass_utils import run_bass_kernel_spmd

B, S, D, R = 4, 2048, 4096, 8
N_CORES = 8
ROWS = B * S                      # 8192
RPC = ROWS // N_CORES             # 1024 rows per core
P = 128
DC = D // P                       # 32 d-chunks
H = RPC // 2                      # 512 rows per half (PSUM bank f32 size)

F32 = mybir.dt.float32
F16 = mybir.dt.float16

_CACHE = {}


def _householder_wy(hra_u: np.ndarray):
    """Return (A, UT) with out = x - (x @ A) @ UT."""
    u = hra_u.astype(np.float32)
    u = u / np.linalg.norm(u, axis=0, keepdims=True)
    U = u.astype(np.float64)
    T = np.zeros((R, R), np.float64)
    for k in range(R):
        T[k, k] = 2.0
        if k:
            T[:k, k] = -2.0 * (T[:k, :k] @ (U[:, :k].T @ U[:, k]))
    A = (U @ T).astype(np.float32)          # [D, R]
    return A, np.ascontiguousarray(u.T)     # [R, D]


# tail drain rotation per 8 chunks: 0=DVE direct psum-sub,
# 1=ACT convert + DVE fp16 sub, 2=ACT convert + Pool fp16 sub
_TAIL = [0, 1, 2, 0, 1, 0, 1, 2]


def _build_program():
    nc = bacc.Bacc(trn_type="TRN2")
    xt = nc.dram_tensor("xt", (D, RPC), F16, kind="ExternalInput")
    a = nc.dram_tensor("a", (P, DC * R), F16, kind="ExternalInput")
    ut = nc.dram_tensor("ut", (R, D), F16, kind="ExternalInput")
    # out is laid out as 2-chunk superlines [(g p), (j r)] so each DMA-out
    # writes 4 KB contiguous per partition (2 KB lines cap ~250 GB/s, 4 KB+
    # sustain ~370-430); the host unpacks.
    out = nc.dram_tensor("out", (D // 2, 2 * RPC), F16, kind="ExternalOutput")

    xtd = xt.rearrange("(c p) r -> p c r", p=P)   # [128, DC, RPC]
    otd = out.rearrange("(g p) (j r) -> p g j r", p=P, j=2)

    with tile.TileContext(nc) as tc:
        with (
            tc.tile_pool(name="const", bufs=1) as const,
            tc.tile_pool(name="upd", bufs=4) as upd_pool,
            tc.tile_pool(name="pso", bufs=4, space="PSUM") as pso_pool,
        ):
            a_sb = const.tile([P, DC * R], F16)
            nc.sync.dma_start(a_sb, a[:, :])
            ut_sb = const.tile([R, D], F16)
            nc.sync.dma_start(ut_sb, ut[:, :])

            xall = const.tile([P, DC, RPC], F16)
            nc.sync.dma_start(xall[:, 0, :], xtd[:, 0, :])

            # tiny PE warm-up: observe each const DMA once (one sync-wait
            # per LDWEIGHTS) and keep the PE awake during the DMA fill
            # without delaying the first projection matmuls.
            warm = pso_pool.tile([P, 2, H], F32, tag="ps_o")
            nc.tensor.matmul(warm[:R, 0, :256], a_sb[:, :R], a_sb[:, :256],
                             start=True, stop=True)
            for _ in range(4):
                nc.tensor.matmul(warm[:, 0, :P], ut_sb[:, :P], ut_sb[:, :P],
                                 start=True, stop=True)

            for c in range(1, DC):
                nc.sync.dma_start(xall[:, c, :], xtd[:, c, :])

            # both partials live in the pso ring; their slots recycle into
            # the tail rotation after the fold, ramping runahead 2 -> 4
            ps_p1 = pso_pool.tile([R, 2, H], F32, tag="ps_o")
            ps_p2 = pso_pool.tile([R, 2, H], F32, tag="ps_o")
            pt = const.tile([R, 2, H], F16)

            # projection: P^T[8, RPC] += A_c^T @ xT_c, accumulated into two
            # alternating PSUM regions (even/odd chunks) so consecutive
            # matmuls never hit the same bank's accumulate turnaround; the
            # two partials are folded by one DVE add at pt time.
            for c in range(DC):
                psp = ps_p1 if c % 2 == 0 else ps_p2
                for h in range(2):
                    nc.tensor.matmul(
                        psp[:, h, :],
                        a_sb[:, c * R:(c + 1) * R],
                        xall[:, c, h * H:(h + 1) * H],
                        start=(c < 2),
                        stop=(c >= DC - 2),
                    )
            # fold per half: the first update matmul (h0) only waits on
            # the h0 fold, shaving ~1us off the phase-boundary bubble
            p2s = const.tile([R, 2, H], F16)
            for h in range(2):
                nc.scalar.copy(p2s[:, h, :], ps_p2[:, h, :])
                nc.vector.tensor_add(pt[:, h, :], ps_p1[:, h, :],
                                     p2s[:, h, :])

            # tail: outT_c = xT_c - U_c @ P^T in place, then DMA out
            for c in range(DC):
                ps_o = pso_pool.tile([P, 2, H], F32, tag="ps_o")
                for h in range(2):
                    nc.tensor.matmul(
                        ps_o[:, h, :],
                        ut_sb[:, c * P:(c + 1) * P],
                        pt[:, h, :],
                        start=True,
                        stop=True,
                    )
                xc = xall[:, c, :]
                kind = _TAIL[c % 8]
                if kind == 0:
                    nc.vector.tensor_sub(xc, xc, ps_o)
                else:
                    u_sb = upd_pool.tile([P, 2, H], F16, tag="upd")
                    nc.scalar.copy(u_sb, ps_o)
                    if kind == 1:
                        nc.vector.tensor_sub(xc, xc, u_sb)
                    else:
                        nc.gpsimd.tensor_sub(xc, xc, u_sb)
                # chunks 0/1 fire solo triggers so the out stream starts
                # ~2us sooner; later chunks flush 2-chunk superlines per
                # 4-block, fastest drain-pairs first, so a slow Pool drain
                # never head-of-line blocks the in-order SP trigger queue
                if c <= 1:
                    nc.sync.dma_start(otd[:, 0, c, :], xall[:, c, :])
                elif c == 3:
                    nc.sync.dma_start(otd[:, 1, :, :], xall[:, 2:4, :])
                elif c % 4 == 3:
                    pairs = [(c - 3) // 2, (c - 1) // 2]
                    pairs.sort(key=lambda g: max(_TAIL[(2 * g) % 8],
                                                 _TAIL[(2 * g + 1) % 8]))
                    for g in pairs:
                        nc.sync.dma_start(otd[:, g, :, :],
                                          xall[:, 2 * g:2 * g + 2, :])

    nc.compile()
    return nc


def _get_program():
    if "nc" not in _CACHE:
        _CACHE["nc"] = _build_program()
    return _CACHE["nc"]


def kernel(input, hra_u, **run_kwargs):
    input = np.asarray(input, dtype=np.float32)
    hra_u = np.asarray(hra_u, dtype=np.float32)

    A, UT = _householder_wy(hra_u)
    # pack A [D, R] so partition p holds A[c*128+p, :] at free offset c*R
    a_packed = np.ascontiguousarray(
        A.reshape(DC, P, R).transpose(1, 0, 2).reshape(P, DC * R)
    ).astype(np.float16)
    ut_f16 = UT.astype(np.float16)

    x_flat = input.reshape(ROWS, D)
    in_maps = [
        {
            "xt": x_flat[c * RPC:(c + 1) * RPC].T.astype(np.float16),
            "a": a_packed,
            "ut": ut_f16,
        }
        for c in range(N_CORES)
    ]

    nc = _get_program()
    res = run_bass_kernel_spmd(nc, in_maps, core_ids=list(range(N_CORES)),
                               **run_kwargs)
    out = np.empty((ROWS, D), dtype=np.float32)
    for c in range(N_CORES):
        o = res.results[c]["out"].reshape(DC // 2, P, 2, RPC)
        out[c * RPC:(c + 1) * RPC] = (
            o.transpose(0, 2, 1, 3).reshape(D, RPC).astype(np.float32).T
        )
    if run_kwargs:
        kernel.last_results = res
    return out.reshape(B, S, D)
